# revision 16
# baseline (speedup 1.0000x reference)
"""Trainium2 Bass kernel for nn_CFGEncoder (3-layer directed GCN + BN + pool).

Self-contained: accepts FULL inputs, shards across 8 NeuronCores internally,
returns the FULL [64, 128] output.

Strategy (per layer, per direction):
  - nodes sharded 12500/core (by scatter target); edges partitioned accordingly
  - node-feature tables stored bf16, padded to 128 dims; gathered-node rows
    fetched with GPSIMD dma_gather (int16 indices into 4 chunk tables of
    25000 rows, 256B rows)
  - segment-sum as one-hot matmuls on PE (bf16): for each 128-edge tile,
    psum[128d, G] += gathered[128e, 128d].T @ onehot[128e, G], with G=128
    node windows accumulated across all 4 chunks in a single PSUM run
    (window-synchronized chunk interleave), flushed once per window to SBUF
    via the ACT engine
  - one-hots built in batches of BT tiles with a single DVE tensor_tensor
  - dense part z = Ws.T@xT + Wi.T@aggin + Wo.T@aggout on PE (bf16 in, fp32
    psum), relu+batchnorm stats on ACT (fp32), global stats via AllReduce
  - h kept feature-major in SBUF between layers (dense rhs of next layer);
    node-major bf16 copy exchanged with an 8-core AllGather for gathers
  - graph mean-pool via one-hot matmul accumulated in one PSUM run
"""
import sys

sys.path.insert(0, "/opt/trn_rl_repo")

import numpy as np

import os as _osmod
NCORE = 8
NCHUNK = 4
G = 128            # scatter window width (nodes per PSUM group)
BT = int(_osmod.environ.get("KBT", "12"))  # 128-edge tiles per dma_gather call
IBC = 8            # gather calls per staged idx block
NQ = 4             # SWDGE queues; queue = chunk index
STRIPE = 512       # dense-phase node stripe
BN_EPS = 1e-5
NUM_GRAPHS = 64
DPAD = 128         # padded feature dim of gather tables


def _bf16():
    import ml_dtypes
    return ml_dtypes.bfloat16


# ----------------------------------------------------------------------------
# host-side preprocessing
# ----------------------------------------------------------------------------

def _edge_streams(gat, sct, N):
    """Build per-core padded gather-index / scatter-loc streams for one
    direction.  gat = node gathered per edge, sct = scatter target per edge.

    Returns dict with:
      idx_wrapped [NCORE, 128, L//16] int16  (dma_gather index layout)
      loc_tiled   [NCORE, 128, L//128] bf16  (one-hot position layout)
      seg_tiles   [NCHUNK, NW] int   (tiles per (chunk, window) segment)
      L           common per-core stream length (multiple of 128)
    """
    S = N // NCORE
    CH = N // NCHUNK
    NW = (S + G - 1) // G
    core = sct // S
    q = gat // CH
    locidx = (gat - q * CH).astype(np.int64)
    w = (sct % S) // G
    lw = (sct % S) % G

    ngrp = NCORE * NCHUNK * NW
    key = (core * NCHUNK + q) * NW + w
    cnt = np.bincount(key, minlength=ngrp).reshape(NCORE, NCHUNK, NW)
    seg_tiles = np.maximum((cnt + 127) // 128, 1).max(axis=0)  # [NCHUNK, NW]
    # per-(q,w) tile counts common to all cores; zero-edge-everywhere
    # segments get dropped
    any_edges = cnt.sum(axis=0) > 0
    seg_tiles = np.where(any_edges, seg_tiles, 0)

    seg_len = seg_tiles * 128                                    # [NCHUNK, NW]
    seg_off = np.zeros(NCHUNK * NW + 1, np.int64)
    np.cumsum(seg_len.reshape(-1), out=seg_off[1:])
    L = int(seg_off[-1])
    assert L % 128 == 0

    order = np.argsort(key, kind="stable")
    skey = key[order]
    # rank of each edge within its (core,q,w) group
    grp_start = np.zeros(ngrp + 1, np.int64)
    np.cumsum(np.bincount(skey, minlength=ngrp), out=grp_start[1:])
    rank = np.arange(len(order)) - grp_start[skey]
    pos = seg_off[(q * NW + w)[order]] + rank

    idx_stream = np.zeros((NCORE, L), np.int16)          # pad -> row 0
    loc_stream = np.full((NCORE, L), -1.0, np.float32)   # pad -> no window hit
    idx_stream[core[order], pos] = locidx[order].astype(np.int16)
    loc_stream[core[order], pos] = lw[order].astype(np.float32)

    idxw = idx_stream.reshape(NCORE, L // 16, 16).transpose(0, 2, 1)  # [NC,16,L/16]
    idx_wrapped = np.tile(idxw, (1, 8, 1)).copy()                     # [NC,128,L/16]
    loc_tiled = (
        loc_stream.reshape(NCORE, L // 128, 128).transpose(0, 2, 1)
        .astype(_bf16()).copy()
    )
    return dict(idx_wrapped=idx_wrapped, loc_tiled=loc_tiled,
                seg_tiles=seg_tiles, L=L, NW=NW)


def _preprocess(x, edge_index, batch):
    N, Din = x.shape
    S = N // NCORE
    bf16 = _bf16()
    src = edge_index[0].astype(np.int64)
    dst = edge_index[1].astype(np.int64)
    stream_in = _edge_streams(src, dst, N)    # aggregate x[src] onto dst
    stream_out = _edge_streams(dst, src, N)   # aggregate x[dst] onto src

    xpad = np.zeros((N, DPAD), bf16)
    xpad[:, :Din] = x.astype(bf16)
    # transposed per-core slices for the dense x@Ws term
    xsT = np.stack([x[c * S:(c + 1) * S].T.astype(bf16) for c in range(NCORE)])

    NT = (S + 127) // 128
    bl = np.full((NCORE, NT * 128), -1.0, np.float32)
    for c in range(NCORE):
        bl[c, :S] = batch[c * S:(c + 1) * S].astype(np.float32)
    bloc = bl.reshape(NCORE, NT, 128).transpose(0, 2, 1).copy()

    cnts = np.bincount(batch.astype(np.int64), minlength=NUM_GRAPHS).astype(np.float32)
    recip = (1.0 / np.maximum(cnts, 1.0)).reshape(NUM_GRAPHS, 1)
    return stream_in, stream_out, xpad, xsT, bloc, recip


# ----------------------------------------------------------------------------
# kernel build
# ----------------------------------------------------------------------------

def _build(nc, tile, mybir, bass, meta):
    """Emit the full 3-layer program into nc (shared by all cores)."""
    N, Din = meta["N"], meta["Din"]
    S = N // NCORE
    CH = N // NCHUNK
    NW = (S + G - 1) // G
    NT = (S + 127) // 128          # node tiles per core slice
    dims = meta["dims"]            # [Din, 128, 128, 128]
    s_in, s_out = meta["s_in"], meta["s_out"]
    AT = mybir.ActivationFunctionType
    f32 = mybir.dt.float32
    bf16 = mybir.dt.bfloat16
    i16 = mybir.dt.int16
    from concourse.masks import make_identity
    from concourse import library_config

    # ---- DRAM I/O ----
    xpad_d = nc.dram_tensor("xpad", [N, DPAD], bf16, kind="ExternalInput")
    xsT_d = nc.dram_tensor("xst", [Din, S], bf16, kind="ExternalInput")
    idx_d, loc_d = {}, {}
    for dname, st in (("in", s_in), ("out", s_out)):
        idx_d[dname] = nc.dram_tensor(
            f"idx_{dname}", [128, st["L"] // 16], i16, kind="ExternalInput")
        loc_d[dname] = nc.dram_tensor(
            f"loc_{dname}", [128, st["L"] // 128], bf16, kind="ExternalInput")
    w_d = {}
    for l in range(3):
        for wn in ("s", "i", "o"):
            w_d[(l, wn)] = nc.dram_tensor(
                f"w{wn}{l}", [dims[l], dims[l + 1]], bf16, kind="ExternalInput")
    gb_d = [nc.dram_tensor(f"gb{l}", [dims[l + 1], 2], f32, kind="ExternalInput")
            for l in range(3)]
    bloc_d = nc.dram_tensor("bloc", [128, NT], f32, kind="ExternalInput")
    recip_d = nc.dram_tensor("recip", [NUM_GRAPHS, 1], f32, kind="ExternalInput")
    out_d = nc.dram_tensor("out", [NUM_GRAPHS, 128], f32, kind="ExternalOutput")

    # internal DRAM
    import os as _os
    _shared = "Local" if _os.environ.get("DBG_LOCAL") else "Shared"
    hfull = [nc.dram_tensor(f"hfull{l}", [N, DPAD], bf16, addr_space=_shared)
             for l in (1, 2)]                       # gather tables for layers 1,2
    hslice = [nc.dram_tensor(f"hslice{l}", [S, DPAD], bf16) for l in (1, 2)]
    bn_in = nc.dram_tensor("bn_in", [128, 2], f32)
    bn_out = nc.dram_tensor("bn_out", [128, 2], f32, addr_space=_shared)
    pool_in = nc.dram_tensor("pool_in", [NUM_GRAPHS, 128], f32)
    pool_out = nc.dram_tensor("pool_out", [NUM_GRAPHS, 128], f32, addr_space=_shared)

    _dbg_null = bool(_os.environ.get("DBG_NULL"))
    _fake_cc = bool(_os.environ.get("DBG_FAKE_CC"))

    def _collective(kind, op, ins, outs):
        if _fake_cc:
            # timing-only stand-in: local copy, wrong values
            nc.sync.dma_start(outs[0].tensor.ap()[:ins[0].shape[0]], ins[0])
        else:
            nc.gpsimd.collective_compute(
                kind, op, replica_groups=[list(range(NCORE))],
                ins=[ins[0].opt()], outs=[outs[0].opt()])

    with tile.TileContext(nc) as tc:
        nc.gpsimd.load_library(library_config.mlp)
        if _dbg_null:
            with tc.tile_pool(name="nul", bufs=1) as nul:
                t = nul.tile([NUM_GRAPHS, 128], f32)
                nc.vector.memset(t[:], 0.0)
                nc.sync.dma_start(out_d[:], t[:])
            return
        import contextlib
        ctx = contextlib.ExitStack()
        const = ctx.enter_context(tc.tile_pool(name="const", bufs=1))
        hp = ctx.enter_context(tc.tile_pool(name="hp", bufs=2))
        aggp = ctx.enter_context(tc.tile_pool(name="aggp", bufs=2))
        gat = ctx.enter_context(tc.tile_pool(name="gat", bufs=8))
        ohp = ctx.enter_context(tc.tile_pool(name="ohp", bufs=8))
        idxp = ctx.enter_context(tc.tile_pool(name="idxp", bufs=8))
        locp = ctx.enter_context(tc.tile_pool(name="locp", bufs=4))
        wrk = ctx.enter_context(tc.tile_pool(name="wrk", bufs=3))
        stg = ctx.enter_context(tc.tile_pool(name="stg", bufs=3))
        scps = ctx.enter_context(tc.tile_pool(name="scps", bufs=2, space="PSUM"))
        dnps = ctx.enter_context(tc.tile_pool(name="dnps", bufs=2, space="PSUM"))
        tpps = ctx.enter_context(tc.tile_pool(name="tpps", bufs=2, space="PSUM"))
        plps = ctx.enter_context(tc.tile_pool(name="plps", bufs=1, space="PSUM"))

        # ---- constants ----
        iota = const.tile([128, G], bf16)
        nc.gpsimd.iota(iota[:], pattern=[[1, G]], base=0, channel_multiplier=0,
                       allow_small_or_imprecise_dtypes=True)
        ident = const.tile([128, 128], bf16)
        make_identity(nc, ident[:])
        w_sb = {}
        for l in range(3):
            for wn in ("s", "i", "o"):
                t = const.tile([dims[l], dims[l + 1]], bf16, name=f"w{wn}{l}_sb")
                nc.sync.dma_start(t[:], w_d[(l, wn)][:])
                w_sb[(l, wn)] = t
        gb_sb = []
        for l in range(3):
            t = const.tile([dims[l + 1], 2], f32, name=f"gb{l}_sb")
            nc.sync.dma_start(t[:], gb_d[l][:])
            gb_sb.append(t)
        recip_sb = const.tile([NUM_GRAPHS, 1], f32)
        nc.sync.dma_start(recip_sb[:], recip_d[:])
        bloc_sb = const.tile([128, NT], f32)
        nc.sync.dma_start(bloc_sb[:], bloc_d[:])
        iota_g = const.tile([128, NUM_GRAPHS], bf16)
        nc.gpsimd.iota(iota_g[:], pattern=[[1, NUM_GRAPHS]], base=0,
                       channel_multiplier=0, allow_small_or_imprecise_dtypes=True)
        # layer-0 dense rhs lives in the h ping-pong pool (freed after dense 0)
        xsT_sb = hp.tile([128, S], bf16, name="xsT_sb", tag="h")
        nc.sync.dma_start(xsT_sb[:Din, :], xsT_d[:])

        # ================= scatter phase =================

        def scatter_direction(l, st, idx_dram, loc_dram, src_dram, agg):
            """Aggregate gathered rows into agg [DPAD, S] (SBUF, bf16).

            Window-synchronized chunk interleave: gather calls are issued
            round-robin across the 4 chunks; each node window w accumulates
            tiles from all 4 chunks into one PSUM run, flushed once."""
            seg_tiles = st["seg_tiles"]          # [NCHUNK, NW]
            import os as _os2
            if _os2.environ.get("DBG_SKIP_SCATTER"):
                nc.vector.memset(agg[:], 0.0)
                return
            _skip_gather = bool(_os2.environ.get("DBG_SKIP_GATHER"))
            _skip_mm = bool(_os2.environ.get("DBG_SKIP_MM"))
            seg_len = seg_tiles * 128
            seg_off = np.zeros(NCHUNK * NW + 1, np.int64)
            np.cumsum(seg_len.reshape(-1), out=seg_off[1:])
            # chunk q's tiles occupy stream tiles [ct0[q], ct0[q+1])
            ct0 = [int(seg_off[q * NW]) // 128 for q in range(NCHUNK + 1)]
            TQ = [ct0[q + 1] - ct0[q] for q in range(NCHUNK)]
            # cumulative tiles within chunk q up to (and excl.) window w
            cum = np.zeros((NCHUNK, NW + 1), np.int64)
            for q in range(NCHUNK):
                np.cumsum(seg_tiles[q], out=cum[q, 1:])

            # per-chunk buffer bookkeeping: list of (tile0_local, gt, oh, nb)
            bufs = [[] for _ in range(NCHUNK)]
            issued = [0] * NCHUNK
            # loc staged per chunk (entire chunk stream); idx staged in blocks
            # of IBC gather calls (BT*IBC tiles) as issuance progresses
            loc_sb = []
            for q in range(NCHUNK):
                if TQ[q] == 0:
                    loc_sb.append(None)
                    continue
                lt = locp.tile([128, TQ[q]], bf16, name=f"loc{l}", tag="loc")
                nc.sync.dma_start(lt[:], loc_dram[:, ct0[q]: ct0[q + 1]])
                loc_sb.append(lt)
            IBT = BT * IBC                    # tiles per idx block
            idx_blk = [None] * NCHUNK         # current staged idx block
            idx_blk_t0 = [0] * NCHUNK         # its first chunk-local tile

            def issue(q):
                b0 = issued[q]
                nb = min(BT, TQ[q] - b0)
                if idx_blk[q] is None or b0 >= idx_blk_t0[q] + IBT:
                    bt0 = (b0 // IBT) * IBT
                    btn = min(IBT, TQ[q] - bt0)
                    it = idxp.tile([128, IBT * 8], i16, name=f"idx{l}", tag="idx")
                    nc.sync.dma_start(
                        it[:, :btn * 8],
                        idx_dram[:, (ct0[q] + bt0) * 8: (ct0[q] + bt0 + btn) * 8])
                    idx_blk[q] = it
                    idx_blk_t0[q] = bt0
                ib0 = b0 - idx_blk_t0[q]
                gt = gat.tile([128, BT, DPAD], bf16, name=f"gt{l}", tag="gt")
                if _skip_gather:
                    nc.vector.memset(gt[:, :nb, :], 0.0)
                else:
                    nc.gpsimd.dma_gather(
                        gt[:, :nb, :], src_dram[q * CH:(q + 1) * CH, :],
                        idx_blk[q][:, ib0 * 8:(ib0 + nb) * 8],
                        nb * 128, nb * 128, DPAD, queue_num=q % NQ)
                oh = ohp.tile([128, BT, G], bf16, name=f"oh{l}", tag="oh")
                nc.vector.tensor_tensor(
                    out=oh[:, :nb, :],
                    in0=loc_sb[q][:, b0:b0 + nb].unsqueeze(2)
                        .broadcast_to([128, nb, G]),
                    in1=iota[:].unsqueeze(1).broadcast_to([128, nb, G]),
                    op=mybir.AluOpType.is_equal)
                bufs[q].append((b0, gt, oh, nb))
                issued[q] = b0 + nb

            def tile_ref(q, t):   # chunk-local tile index -> (gt, oh, slot)
                for (b0, gt, oh, nb) in reversed(bufs[q]):
                    if b0 <= t < b0 + nb:
                        return gt, oh, t - b0
                raise AssertionError("tile not issued")

            w_done = 0
            while w_done < NW:
                for q in range(NCHUNK):
                    if issued[q] < TQ[q]:
                        issue(q)
                while w_done < NW:
                    w = w_done
                    if not all(cum[q, w + 1] <= issued[q] for q in range(NCHUNK)):
                        break
                    gw = min(G, S - w * G)
                    ntw_tot = int(cum[:, w + 1].sum() - cum[:, w].sum())
                    if ntw_tot == 0 or _skip_mm:
                        nc.vector.memset(agg[:, w * G: w * G + gw], 0.0)
                        w_done += 1
                        continue
                    ps = scps.tile([DPAD, G], f32, space="PSUM",
                                   name=f"scps{l}", tag="sc")
                    k = 0
                    for q in range(NCHUNK):
                        for t in range(int(cum[q, w]), int(cum[q, w + 1])):
                            gt, oh, slot = tile_ref(q, t)
                            nc.tensor.matmul(
                                out=ps[:], lhsT=gt[:, slot, :], rhs=oh[:, slot, :],
                                start=(k == 0), stop=(k == ntw_tot - 1))
                            k += 1
                    nc.scalar.copy(agg[:, w * G: w * G + gw], ps[:, :gw])
                    w_done += 1

        # ============== dense + BN ==============
        def dense_layer(l, d_in, d_out, rhs_x, agg_in, agg_out):
            """zbuf[d_out, S] (bf16) = relu(Ws.T x + Wi.T agg_in + Wo.T agg_out);
            BN stats -> returns (zbuf, scale, bias)."""
            nstripe = (S + STRIPE - 1) // STRIPE
            zbuf = hp.tile([128, S], bf16, name=f"z{l}", tag="h")
            stats = wrk.tile([d_out, 2 * nstripe], f32, name=f"stats{l}", tag="stats")
            sq = wrk.tile([d_out, STRIPE], bf16, name=f"sq{l}", tag="sq")
            for s in range(nstripe):
                n0 = s * STRIPE
                ns = min(STRIPE, S - n0)
                ps = dnps.tile([d_out, STRIPE], f32, space="PSUM",
                               name=f"dn{l}", tag="dn")
                nc.tensor.matmul(out=ps[:, :ns], lhsT=w_sb[(l, "s")][:],
                                 rhs=rhs_x[:d_in, n0:n0 + ns],
                                 start=True, stop=False)
                nc.tensor.matmul(out=ps[:, :ns], lhsT=w_sb[(l, "i")][:],
                                 rhs=agg_in[:d_in, n0:n0 + ns],
                                 start=False, stop=False)
                nc.tensor.matmul(out=ps[:, :ns], lhsT=w_sb[(l, "o")][:],
                                 rhs=agg_out[:d_in, n0:n0 + ns],
                                 start=False, stop=True)
                nc.scalar.activation(out=zbuf[:d_out, n0:n0 + ns], in_=ps[:, :ns],
                                     func=AT.Relu,
                                     accum_out=stats[:, s:s + 1])
                nc.scalar.activation(out=sq[:, :ns], in_=zbuf[:d_out, n0:n0 + ns],
                                     func=AT.Square,
                                     accum_out=stats[:, nstripe + s:nstripe + s + 1])
            loc_sums = wrk.tile([d_out, 2], f32, name=f"bnsum{l}")
            nc.vector.tensor_reduce(out=loc_sums[:, 0:1], in_=stats[:, :nstripe],
                                    axis=mybir.AxisListType.X, op=mybir.AluOpType.add)
            nc.vector.tensor_reduce(out=loc_sums[:, 1:2],
                                    in_=stats[:, nstripe:2 * nstripe],
                                    axis=mybir.AxisListType.X, op=mybir.AluOpType.add)
            nc.sync.dma_start(bn_in[:d_out, :], loc_sums[:])
            _collective("AllReduce", mybir.AluOpType.add, [bn_in[:]], [bn_out[:]])
            gsum = wrk.tile([d_out, 2], f32, name=f"bng{l}")
            nc.sync.dma_start(gsum[:], bn_out[:d_out, :])
            # mu = gsum0/N ; ex2 = gsum1/N ; var = ex2 - mu^2
            mu = wrk.tile([d_out, 1], f32, name=f"mu{l}")
            var = wrk.tile([d_out, 1], f32, name=f"var{l}")
            nc.vector.tensor_scalar(out=mu[:], in0=gsum[:, 0:1], scalar1=1.0 / N,
                                    scalar2=None, op0=mybir.AluOpType.mult)
            nc.vector.tensor_scalar(out=var[:], in0=gsum[:, 1:2], scalar1=1.0 / N,
                                    scalar2=None, op0=mybir.AluOpType.mult)
            mu2 = wrk.tile([d_out, 1], f32, name=f"mu2{l}")
            nc.vector.tensor_tensor(out=mu2[:], in0=mu[:], in1=mu[:],
                                    op=mybir.AluOpType.mult)
            nc.vector.tensor_tensor(out=var[:], in0=var[:], in1=mu2[:],
                                    op=mybir.AluOpType.subtract)
            sd = wrk.tile([d_out, 1], f32, name=f"sd{l}")
            nc.vector.tensor_scalar(out=sd[:], in0=var[:], scalar1=BN_EPS,
                                    scalar2=None, op0=mybir.AluOpType.add)
            nc.scalar.sqrt(sd[:], sd[:])
            rs = wrk.tile([d_out, 1], f32, name=f"rs{l}")
            nc.vector.reciprocal(rs[:], sd[:])
            scale = wrk.tile([d_out, 1], f32, name=f"scale{l}")
            nc.vector.tensor_tensor(out=scale[:], in0=gb_sb[l][:, 0:1], in1=rs[:],
                                    op=mybir.AluOpType.mult)
            bias = wrk.tile([d_out, 1], f32, name=f"bias{l}")
            nc.vector.tensor_tensor(out=bias[:], in0=mu[:], in1=scale[:],
                                    op=mybir.AluOpType.mult)
            nc.vector.tensor_tensor(out=bias[:], in0=gb_sb[l][:, 1:2], in1=bias[:],
                                    op=mybir.AluOpType.subtract)
            return zbuf, scale, bias

        # ============== per-layer driver ==============
        gather_src = [xpad_d, hfull[0], hfull[1]]   # full-table gather sources
        rhs_x = xsT_sb
        pool_ps = None
        for l in range(3):
            d_in, d_out = dims[l], dims[l + 1]
            agg_in = aggp.tile([DPAD, S], bf16, name=f"aggin{l}", tag="agg")
            scatter_direction(l, s_in, idx_d["in"], loc_d["in"],
                              gather_src[l], agg_in)
            agg_out = aggp.tile([DPAD, S], bf16, name=f"aggout{l}", tag="agg")
            scatter_direction(l, s_out, idx_d["out"], loc_d["out"],
                              gather_src[l], agg_out)
            zbuf, scale, bias = dense_layer(l, d_in, d_out, rhs_x, agg_in, agg_out)

            # apply BN in place (feature-major): h = scale*z + bias
            nstripe = (S + STRIPE - 1) // STRIPE
            for s in range(nstripe):
                n0 = s * STRIPE
                ns = min(STRIPE, S - n0)
                nc.scalar.activation(out=zbuf[:d_out, n0:n0 + ns],
                                     in_=zbuf[:d_out, n0:n0 + ns],
                                     func=AT.Identity, scale=scale[:], bias=bias[:])
            rhs_x = zbuf

            # node-major production: transpose tiles; layers 0-1 write
            # hslice + AllGather, layer 2 feeds pooling.
            SBT = 4      # tiles per staged DMA write
            for t0 in range(0, NT, SBT):
                nts = min(SBT, NT - t0)
                hst = stg.tile([128, SBT * 128], bf16, name=f"hst{l}", tag="hst")
                for j in range(nts):
                    t = t0 + j
                    c0 = t * 128
                    tn = min(128, S - c0)
                    tp = tpps.tile([128, 128], bf16, space="PSUM",
                                   name=f"htp{l}", tag="tp")
                    nc.tensor.transpose(out=tp[:tn, :], in_=zbuf[:, c0:c0 + tn],
                                        identity=ident[:])
                    nc.scalar.copy(hst[:tn, j * 128:(j + 1) * 128], tp[:tn, :128])
                    if l == 2:
                        goh = stg.tile([128, NUM_GRAPHS], bf16, name="goh", tag="goh")
                        nc.vector.tensor_scalar(
                            out=goh[:tn, :], in0=iota_g[:tn, :],
                            scalar1=bloc_sb[:tn, t:t + 1], scalar2=None,
                            op0=mybir.AluOpType.is_equal)
                        if pool_ps is None:
                            pool_ps = plps.tile([NUM_GRAPHS, 128], f32,
                                                space="PSUM", name="poolps")
                        nc.tensor.matmul(out=pool_ps[:], lhsT=goh[:tn, :],
                                         rhs=hst[:tn, j * 128:(j + 1) * 128],
                                         start=(t == 0), stop=(t == NT - 1))
                if l < 2:
                    r0 = t0 * 128
                    if r0 + nts * 128 <= S:
                        # full 128-row tiles: partition-outer DRAM AP
                        out_ap = (hslice[l][r0: r0 + nts * 128, :]
                                  .rearrange("(t p) q -> p t q", p=128))
                        in_ap = (hst[:, :nts * 128]
                                 .rearrange("p (t q) -> p t q", q=128))
                        nc.sync.dma_start(out_ap, in_ap)
                    else:
                        for j in range(nts):
                            t = t0 + j
                            tn = min(128, S - t * 128)
                            nc.sync.dma_start(
                                hslice[l][t * 128: t * 128 + tn, :],
                                hst[:tn, j * 128:(j + 1) * 128])
            if l < 2:
                _collective("AllGather", mybir.AluOpType.bypass,
                            [hslice[l][:]], [hfull[l][:]])

        # ============== pooling reduce ==============
        pool_sb = wrk.tile([NUM_GRAPHS, 128], f32, name="pool_sb")
        nc.scalar.copy(pool_sb[:], pool_ps[:])
        nc.sync.dma_start(pool_in[:], pool_sb[:])
        _collective("AllReduce", mybir.AluOpType.add, [pool_in[:]], [pool_out[:]])
        pool_g = wrk.tile([NUM_GRAPHS, 128], f32, name="pool_g")
        nc.sync.dma_start(pool_g[:], pool_out[:])
        res = wrk.tile([NUM_GRAPHS, 128], f32, name="res")
        nc.vector.tensor_scalar(out=res[:], in0=pool_g[:], scalar1=recip_sb[:],
                                scalar2=None, op0=mybir.AluOpType.mult)
        nc.sync.dma_start(out_d[:], res[:])
        ctx.close()


# ----------------------------------------------------------------------------
# public entry
# ----------------------------------------------------------------------------

_CACHE = {}


def _pjrt_runner(nc, in_maps, out_names_shapes):
    """Build a reusable sharded-jit executable for the SPMD program."""
    import jax
    from jax.sharding import Mesh, PartitionSpec, NamedSharding
    from jax.experimental.shard_map import shard_map
    from concourse import bass2jax

    bass2jax.install_neuronx_cc_hook()
    in_names = list(in_maps[0].keys())
    out_names = [n for n, _ in out_names_shapes]
    out_avals = [jax.core.ShapedArray(s, np.float32)
                 for _, s in out_names_shapes]
    pname = nc.partition_id_tensor.name if nc.partition_id_tensor else None
    all_in = in_names + out_names + ([pname] if pname else [])

    def _body(*args):
        ops = list(args)
        if pname is not None:
            ops.append(bass2jax.partition_id_tensor())
        return tuple(bass2jax._bass_exec_p.bind(
            *ops, out_avals=tuple(out_avals), in_names=tuple(all_in),
            out_names=tuple(out_names),
            lowering_input_output_aliases=(), sim_require_finite=True,
            sim_require_nnan=True, nc=nc))

    devices = jax.devices()[:NCORE]
    mesh = Mesh(np.asarray(devices), ("core",))
    nin = len(in_names) + len(out_names)
    fn = jax.jit(shard_map(_body, mesh=mesh,
                           in_specs=(PartitionSpec("core"),) * nin,
                           out_specs=(PartitionSpec("core"),) * len(out_names),
                           check_rep=False), keep_unused=True)
    sh = NamedSharding(mesh, PartitionSpec("core"))
    dev_in = [jax.device_put(
        np.concatenate([np.asarray(m[n]) for m in in_maps], axis=0), sh)
        for n in in_names]
    dev_zero = [jax.device_put(
        np.zeros((NCORE * s[0],) + tuple(s[1:]), np.float32), sh)
        for _, s in out_names_shapes]

    def run():
        outs = fn(*dev_in, *dev_zero)
        jax.block_until_ready(outs)
        return {n: np.asarray(outs[i]).reshape((NCORE,) + tuple(out_avals[i].shape))
                for i, n in enumerate(out_names)}

    def run_nofetch():
        outs = fn(*dev_in, *dev_zero)
        jax.block_until_ready(outs)

    def run_trace():
        import gauge.profiler
        with gauge.profiler.profile(
                kernel_dev_mode=True, profile_on_exit=False,
                bass_kernel=nc.m) as profile:
            jax.block_until_ready(fn(*dev_in, *dev_zero))
        print("PROFILE_PATH:", profile.profile_path)
        try:
            pres = profile.to_perfetto(model_index="all")
            for p in pres or []:
                print("PERFETTO:", getattr(p, "path", p))
        except Exception as e:
            print("to_perfetto failed:", e)
        return profile

    run.nofetch = run_nofetch
    run.trace = run_trace
    return run


def _make_runner(x, edge_index, batch, weights):
    import concourse.bacc as bacc
    import concourse.bass as bass
    import concourse.tile as tile
    from concourse import mybir

    bf16 = _bf16()
    N, Din = x.shape
    S = N // NCORE
    s_in, s_out, xpad, xsT, bloc, recip = _preprocess(x, edge_index, batch)
    dims = [Din] + [weights[f"Ws{i}"].shape[1] for i in range(3)]

    nc = bacc.Bacc("TRN2", target_bir_lowering=False, debug=False,
                   num_devices=NCORE, num_swdge_queues=NQ)
    meta = dict(N=N, Din=Din, dims=dims, s_in=s_in, s_out=s_out)
    _build(nc, tile, mybir, bass, meta)
    nc.compile()

    in_maps = []
    for c in range(NCORE):
        m = {
            "xpad": xpad,
            "xst": xsT[c],
            "idx_in": s_in["idx_wrapped"][c],
            "loc_in": s_in["loc_tiled"][c],
            "idx_out": s_out["idx_wrapped"][c],
            "loc_out": s_out["loc_tiled"][c],
            "bloc": bloc[c],
            "recip": recip,
        }
        for i in range(3):
            m[f"ws{i}"] = weights[f"Ws{i}"].astype(bf16)
            m[f"wi{i}"] = weights[f"Wi{i}"].astype(bf16)
            m[f"wo{i}"] = weights[f"Wo{i}"].astype(bf16)
            m[f"gb{i}"] = np.stack(
                [weights[f"g{i}"], weights[f"b{i}"]], axis=1).astype(np.float32)
        in_maps.append(m)

    import os
    if os.environ.get("KERNEL_SIM"):
        from concourse.bass_interp import MultiCoreSim

        def run():
            sim = MultiCoreSim(nc, num_cores=NCORE, trace=False,
                               require_finite=False, require_nnan=False)
            for c, core in sim.cores.items():
                for k, v in in_maps[c].items():
                    core.tensor(k)[:] = v
            sim.simulate(check_with_hw=False)

            class R:
                results = [{"out": np.array(sim.cores[c].tensor("out"))}
                           for c in range(NCORE)]
            return R()

        return run

    runner = _pjrt_runner(nc, in_maps, [("out", (NUM_GRAPHS, 128))])

    def run():
        outs = runner()

        class R:
            results = [{"out": outs["out"][c]} for c in range(NCORE)]
        return R()

    run.nofetch = runner.nofetch
    run.trace = getattr(runner, "trace", None)
    return run


def kernel(x, edge_index, batch, **weights):
    x = np.asarray(x, dtype=np.float32)
    edge_index = np.asarray(edge_index)
    batch = np.asarray(batch)
    weights = {k: np.asarray(v, dtype=np.float32) for k, v in weights.items()}
    run = _make_runner(x, edge_index, batch, weights)
    res = run()
    return np.asarray(res.results[0]["out"], dtype=np.float32)
